# revision 12
# baseline (speedup 1.0000x reference)
"""Trainium2 Bass kernel for nn_CentroidDistance (Lorentz/hyperbolic KNN distances).

Computes: dist[n, c] = arccosh(max(-<node_n, cent_c>_Lorentz, 1+eps)) * mask[n]
where cent = hyp_linear(expmap0(proj_tan0(centroid_weight)), W, b).

Sharding: data-parallel over the 65536 node rows across 8 NeuronCores; the
small centroid table / W / b are replicated.  Each core computes an
[8192, 1024] block of the output independently (no collectives).

Device pipeline per core:
  prep (tiny): build the transformed centroid table c_hat^T [64, 1024] on-chip,
    where c_hat = [c0, -c_spatial] so that  x := node . c_hat = -<node,c>_L.
  main loop over 64 node tiles of 128 rows:
    PE   : x = node_tile^T . c_hatT          (PSUM, 2 banks)
    DVE  : z = x*x                           (PSUM -> SBUF)   [split with ACT]
    ACT  : s = sqrt(z - 1)                   (sqrt table set)
    DVE  : t = x + s
    ACT  : d = ln(t)  ( = arccosh(x) )       (ln table set)
    DMA  : d -> HBM
  ACT table sets are phase-batched per chunk of tiles to avoid table thrash.
"""

import os
import numpy as np

import concourse.bass as bass
import concourse.bacc as bacc
import concourse.tile as tile
from concourse import mybir
from concourse.bass_utils import run_bass_kernel_spmd
from concourse.masks import make_identity
from concourse.tile import add_dep_helper

AF = mybir.ActivationFunctionType
ALU = mybir.AluOpType
F32 = mybir.dt.float32

N_CORES = 8
NODE_NUM = 65536
C = 1024
D = 64
SHARD = NODE_NUM // N_CORES          # 8192 nodes per core
NTILES = SHARD // 128                # 64 tiles of 128 nodes
EPS = 1e-6

# ---- tunables ----
CHUNK = 22          # node-tiles per ACT table phase (must be even)
SQ_ACT_EVERY = 12   # every Nth pair's square runs on ACT instead of GpSimd
MM_DTYPE = "f32r"   # "f32" | "f32r"

LAST_EXEC_TIME_NS = None
_PROGRAMS = {}


def _register_const(nc, val):
    t = nc.alloc_sbuf_tensor(f"const-f32-{val}", [128, 1], F32)
    nc.gpsimd.memset(t.ap(), val)
    nc.const_aps.aps[(F32, val)] = t.ap()


def _build(apply_mask: bool, clamp: bool) -> bass.Bass:
    nc = bacc.Bacc("TRN2")
    _register_const(nc, -1.0)
    nc.all_engine_barrier()

    mm_dt = F32 if MM_DTYPE == "f32" else mybir.dt.float32r

    node_p = nc.dram_tensor("node_p", [128, SHARD // 2], mm_dt, kind="ExternalInput")
    cw = nc.dram_tensor("cw", [C, D], F32, kind="ExternalInput")
    wt = nc.dram_tensor("wt", [D, D], F32, kind="ExternalInput")
    bvec = nc.dram_tensor("bvec", [1, D], F32, kind="ExternalInput")
    if apply_mask:
        maskc = nc.dram_tensor("maskc", [128, NTILES], F32, kind="ExternalInput")
    dist = nc.dram_tensor("dist", [SHARD, C], F32, kind="ExternalOutput")

    with tile.TileContext(nc) as tc:
        from contextlib import ExitStack

        with ExitStack() as outer:
            singles = outer.enter_context(tc.tile_pool(name="singles", bufs=1))

            # ---- persistent tiles ----
            node_sb = singles.tile([128, SHARD // 2], mm_dt)    # 16 KiB/part
            cT = singles.tile([128, C], mm_dt)                  # c_hat^T, duplicated at part 64
            ident = singles.tile([128, 128], F32)
            wt_sb = singles.tile([D, D], F32)
            b_bc = singles.tile([128, D], F32)
            if apply_mask:
                mask_sb = singles.tile([128, NTILES], F32)

            nc.sync.dma_start(out=node_sb, in_=node_p[:, :])
            nc.sync.dma_start(out=wt_sb, in_=wt[:, :])
            nc.gpsimd.dma_start(
                out=b_bc, in_=bass.AP(tensor=bvec, offset=0, ap=[[0, 128], [1, D]])
            )
            if apply_mask:
                nc.sync.dma_start(out=mask_sb, in_=maskc[:, :])
            make_identity(nc, ident)

            # ================= centroid prep =================
            with ExitStack() as prep:
                pp = prep.enter_context(tc.tile_pool(name="prep", bufs=1))
                pps = prep.enter_context(
                    tc.tile_pool(name="prep_ps", bufs=2, space="PSUM")
                )

                cw_all = pp.tile([128, 8, D], F32)
                nc.sync.dma_start(
                    out=cw_all, in_=cw[:, :].rearrange("(r p) d -> p r d", p=128)
                )

                sq = pp.tile([128, 8, D - 1], F32)
                nc.vector.tensor_mul(sq, cw_all[:, :, 1:], cw_all[:, :, 1:])
                nrm2 = pp.tile([128, 8], F32)
                nc.vector.tensor_reduce(
                    nrm2, sq, axis=mybir.AxisListType.X, op=ALU.add
                )
                nrm2c = pp.tile([128, 8], F32)
                nc.vector.tensor_scalar_max(nrm2c, nrm2, EPS)
                # n = sqrt(nrm2c) = exp(0.5*ln(nrm2c)); keeps prep on one table set
                lg = pp.tile([128, 8], F32)
                nc.scalar.activation(lg, nrm2c, AF.Ln)
                nvec = pp.tile([128, 8], F32)
                nc.scalar.activation(nvec, lg, AF.Exp, scale=0.5)
                e1 = pp.tile([128, 8], F32)
                nc.scalar.activation(e1, nvec, AF.Exp)
                e2 = pp.tile([128, 8], F32)
                nc.scalar.activation(e2, nvec, AF.Exp, scale=-1.0)
                coshn = pp.tile([128, 8], F32)
                nc.vector.tensor_add(coshn, e1, e2)
                nc.vector.tensor_scalar_mul(coshn, coshn, 0.5)
                rn = pp.tile([128, 8], F32)
                nc.vector.reciprocal(rn, nvec)
                sdiff = pp.tile([128, 8], F32)
                nc.vector.tensor_sub(sdiff, e1, e2)
                fall = pp.tile([128, 8], F32)
                # fall = (0.5 * sdiff) * rn  == sinh(n)/n
                nc.vector.scalar_tensor_tensor(
                    fall, sdiff, 0.5, rn, op0=ALU.mult, op1=ALU.mult
                )

                y_all = pp.tile([128, 8, D], F32)
                t0_in = pp.tile([128, 8], F32)
                for r in range(8):
                    pt = pp.tile([128, D], F32, tag="pt")
                    nc.vector.tensor_copy(pt[:, 0:1], coshn[:, r : r + 1])
                    nc.vector.tensor_scalar_mul(
                        pt[:, 1:], cw_all[:, r, 1:], fall[:, r : r + 1]
                    )
                    ptT_ps = pps.tile([64, 128], F32, tag="ptT_ps")
                    nc.tensor.transpose(ptT_ps, pt, ident)
                    ptT = pp.tile([64, 128], F32, tag="ptT")
                    nc.vector.tensor_copy(ptT, ptT_ps)
                    y_ps = pps.tile([128, D], F32, tag="y_ps")
                    nc.tensor.matmul(y_ps, ptT, wt_sb, start=True, stop=True)
                    nc.vector.tensor_add(y_all[:, r, :], y_ps, b_bc)

                sq2 = pp.tile([128, 8, D - 1], F32)
                nc.vector.tensor_mul(sq2, y_all[:, :, 1:], y_all[:, :, 1:])
                s2 = pp.tile([128, 8], F32)
                nc.vector.tensor_reduce(s2, sq2, axis=mybir.AxisListType.X, op=ALU.add)
                # t0 = sqrt(1 + s2) = exp(0.5*ln(s2 + 1))
                nc.scalar.activation(t0_in, s2, AF.Ln, bias=1.0)
                t0 = pp.tile([128, 8], F32)
                nc.scalar.activation(t0, t0_in, AF.Exp, scale=0.5)

                ch_all = pp.tile([128, 8, D], F32)
                nc.vector.tensor_copy(ch_all[:, :, 0:1], t0)
                nc.vector.tensor_scalar_mul(ch_all[:, :, 1:], y_all[:, :, 1:], -1.0)
                # stage all 8 transposed blocks in one PSUM region, then land
                # cT with a single copy so downstream matmuls wait on one inst
                chT_all = pps.tile([64, 8, 128], F32, tag="chT_all")
                for r in range(8):
                    nc.tensor.transpose(chT_all[:, r, :], ch_all[:, r, :], ident)
                nc.vector.tensor_copy(cT[0:64, :], chT_all)
                # duplicate c_hat^T into partitions 64..127 so matmuls for the
                # second half of the node slab see matching base partitions
                nc.sync.dma_start(out=cT[64:128, :], in_=cT[0:64, :])

            # ================= main loop =================
            # per tile: PE mm -> x (PSUM); DVE: xe = max(x, 1+eps) (clamp +
            # eviction to SBUF); square on GpSimd (mostly) / ACT (some pairs);
            # ACT: s = sqrt(z-1); DVE: t = xe + s; ACT: d = ln(t); DMA out.
            with ExitStack() as main:
                xs = main.enter_context(
                    tc.tile_pool(name="x_ps", bufs=3, space="PSUM")
                )
                xes = main.enter_context(tc.tile_pool(name="xes", bufs=3))
                zs = main.enter_context(tc.tile_pool(name="zs", bufs=2))
                ss = main.enter_context(tc.tile_pool(name="ss", bufs=2))
                ts_pool = main.enter_context(
                    tc.tile_pool(name="ts", bufs=max(2, CHUNK // 2))
                )
                os_pool = main.enter_context(tc.tile_pool(name="os", bufs=2))
                if apply_mask:
                    ds_pool = main.enter_context(tc.tile_pool(name="ds", bufs=2))

                dist_v = dist[:, :].rearrange("(a b p) c -> a p b c", b=2, p=128)

                last_ln = None
                i0 = 0
                npair = 0
                while i0 < NTILES:
                    nch = min(CHUNK, NTILES - i0)
                    tpairs = []
                    first_q = None
                    last_q = None
                    for j in range(nch):
                        i = i0 + j
                        half, col = (0, i * 128) if i < 32 else (64, (i - 32) * 128)
                        lhsT = node_sb[half : half + 64, col : col + 128]

                        x_ps = xs.tile([128, C], F32, tag="x")
                        for bk in range(2):
                            nc.tensor.matmul(
                                x_ps[:, bk * 512 : (bk + 1) * 512],
                                lhsT,
                                cT[half : half + 64, bk * 512 : (bk + 1) * 512],
                                start=True,
                                stop=True,
                            )

                        if j % 2 == 0:
                            xe_pair = xes.tile([128, 2, C], F32, tag="xe")
                            z_pair = zs.tile([128, 2, C], F32, tag="z")
                            s_pair = ss.tile([128, 2, C], F32, tag="s")
                            t_pair = ts_pool.tile([128, 2, C], F32, tag="t")
                            tpairs.append((t_pair, i))

                        # clamp + evict PSUM -> SBUF in one DVE pass
                        nc.vector.tensor_scalar_max(
                            xe_pair[:, j % 2, :], x_ps, 1.0 + EPS
                        )

                        if j % 2 == 1:
                            xf = xe_pair.rearrange("p a c -> p (a c)")
                            zf = z_pair.rearrange("p a c -> p (a c)")
                            sf = s_pair.rearrange("p a c -> p (a c)")
                            tf = t_pair.rearrange("p a c -> p (a c)")
                            if npair % SQ_ACT_EVERY == SQ_ACT_EVERY - 1:
                                qs = nc.scalar.activation(zf, xf, AF.Square)
                                if first_q is None:
                                    first_q = qs
                            else:
                                nc.gpsimd.tensor_mul(zf, xf, xf)
                            qi = nc.scalar.activation(sf, zf, AF.Sqrt, bias=-1.0)
                            if first_q is None:
                                first_q = qi
                            last_q = qi
                            nc.vector.tensor_add(tf, xf, sf)
                            npair += 1

                    if last_ln is not None:
                        # keep ACT in sqrt-phase order after previous ln-phase
                        add_dep_helper(first_q.ins, last_ln.ins, sync=False)

                    for t_pair, i in tpairs:
                        pair = i // 2
                        o2 = os_pool.tile([128, 2, C], F32, tag="o")
                        if apply_mask:
                            d2 = ds_pool.tile([128, 2, C], F32, tag="d")
                            li = nc.scalar.activation(
                                d2.rearrange("p a c -> p (a c)"),
                                t_pair.rearrange("p a c -> p (a c)"),
                                AF.Ln,
                            )
                            for h in range(2):
                                nc.gpsimd.tensor_scalar_mul(
                                    o2[:, h, :],
                                    d2[:, h, :],
                                    mask_sb[:, i + h : i + h + 1],
                                )
                        else:
                            li = nc.scalar.activation(
                                o2.rearrange("p a c -> p (a c)"),
                                t_pair.rearrange("p a c -> p (a c)"),
                                AF.Ln,
                            )
                        add_dep_helper(li.ins, last_q.ins, sync=False)
                        last_ln = li
                        nc.sync.dma_start(out=dist_v[pair], in_=o2)

                    i0 += nch

    nc.finalize()
    return nc


def _get_program(apply_mask: bool, clamp: bool) -> bass.Bass:
    key = (apply_mask, clamp, CHUNK, SQ_ACT_EVERY, MM_DTYPE)
    if key not in _PROGRAMS:
        _PROGRAMS[key] = _build(apply_mask, clamp)
    return _PROGRAMS[key]


def _round_f32r(x):
    import ml_dtypes

    hi = x.astype(ml_dtypes.bfloat16).astype(np.float32)
    lo = (x - hi).astype(ml_dtypes.bfloat16).astype(np.float32)
    return (hi + lo).astype(np.float32)


def kernel(node_repr, mask, centroid_weight, W, b):
    global LAST_EXEC_TIME_NS

    node = np.ascontiguousarray(np.asarray(node_repr, dtype=np.float32))
    if MM_DTYPE == "f32r":
        node = _round_f32r(node)
    mask_np = np.ascontiguousarray(np.asarray(mask, dtype=np.float32)).reshape(
        NODE_NUM, 1
    )
    cw_np = np.ascontiguousarray(np.asarray(centroid_weight, dtype=np.float32))
    w_np = np.asarray(W, dtype=np.float32)
    b_np = np.ascontiguousarray(np.asarray(b, dtype=np.float32)).reshape(1, D)
    wt_np = np.ascontiguousarray(w_np.T)

    apply_mask = not bool(np.all(mask_np == 1.0))

    nc = _get_program(apply_mask, False)

    in_maps = []
    for k in range(N_CORES):
        nt = node[k * SHARD : (k + 1) * SHARD, :].T  # [64, 8192]
        node_p = np.ascontiguousarray(
            np.concatenate([nt[:, : SHARD // 2], nt[:, SHARD // 2 :]], axis=0)
        )
        im = {"node_p": node_p, "cw": cw_np, "wt": wt_np, "bvec": b_np}
        if apply_mask:
            im["maskc"] = np.ascontiguousarray(
                mask_np[k * SHARD : (k + 1) * SHARD, 0].reshape(NTILES, 128).T
            )
        in_maps.append(im)

    trace = bool(int(os.environ.get("CD_TRACE", "0")))
    res = run_bass_kernel_spmd(nc, in_maps, list(range(N_CORES)), trace=trace)
    LAST_EXEC_TIME_NS = res.exec_time_ns

    out = np.concatenate([r["dist"] for r in res.results], axis=0)
    return out.astype(np.float32, copy=False)


# revision 15
# speedup vs baseline: 1.4148x; 1.4148x over previous
"""Trainium2 Bass kernel for nn_CentroidDistance (Lorentz/hyperbolic KNN distances).

Computes: dist[n, c] = arccosh(max(-<node_n, cent_c>_Lorentz, 1+eps)) * mask[n]
where cent = hyp_linear(expmap0(proj_tan0(centroid_weight)), W, b).

Sharding: data-parallel over the 65536 node rows across 8 NeuronCores; the
small centroid table / W / b are replicated.  Each core computes an
[8192, 1024] block of the output independently (no collectives).

Device pipeline per core:
  prep (tiny): build the transformed centroid table c_hat^T [64, 1024] on-chip,
    where c_hat = [c0, -c_spatial] so that  x := node . c_hat = -<node,c>_L.
  main loop over 64 node tiles of 128 rows:
    PE   : x = node_tile^T . c_hatT          (PSUM, 2 banks)
    DVE  : z = x*x                           (PSUM -> SBUF)   [split with ACT]
    ACT  : s = sqrt(z - 1)                   (sqrt table set)
    DVE  : t = x + s
    ACT  : d = ln(t)  ( = arccosh(x) )       (ln table set)
    DMA  : d -> HBM
  ACT table sets are phase-batched per chunk of tiles to avoid table thrash.
"""

import os
import numpy as np

import concourse.bass as bass
import concourse.bacc as bacc
import concourse.tile as tile
from concourse import mybir
from concourse.bass_utils import run_bass_kernel_spmd
from concourse.masks import make_identity
from concourse.tile import add_dep_helper

AF = mybir.ActivationFunctionType
ALU = mybir.AluOpType
F32 = mybir.dt.float32

N_CORES = 8
NODE_NUM = 65536
C = 1024
D = 64
SHARD = NODE_NUM // N_CORES          # 8192 nodes per core
NTILES = SHARD // 128                # 64 tiles of 128 nodes
EPS = 1e-6

# ---- tunables ----
CHUNK = 24          # node-tiles per ACT table phase (multiple of 4)
SQ_ACT_EVERY = 12   # every Nth pair's square runs on ACT instead of GpSimd
MM_DTYPE = "f32r"   # "f32" | "f32r"

LAST_EXEC_TIME_NS = None
_PROGRAMS = {}


def _register_const(nc, val):
    t = nc.alloc_sbuf_tensor(f"const-f32-{val}", [128, 1], F32)
    nc.gpsimd.memset(t.ap(), val)
    nc.const_aps.aps[(F32, val)] = t.ap()


def _build(apply_mask: bool, clamp: bool) -> bass.Bass:
    nc = bacc.Bacc("TRN2")
    _register_const(nc, -1.0)
    nc.all_engine_barrier()

    mm_dt = F32 if MM_DTYPE == "f32" else mybir.dt.float32r

    node_p = nc.dram_tensor("node_p", [128, SHARD // 2], mm_dt, kind="ExternalInput")
    cw = nc.dram_tensor("cw", [C, D], F32, kind="ExternalInput")
    wt = nc.dram_tensor("wt", [D, D], F32, kind="ExternalInput")
    bvec = nc.dram_tensor("bvec", [1, D], F32, kind="ExternalInput")
    if apply_mask:
        maskc = nc.dram_tensor("maskc", [128, NTILES], F32, kind="ExternalInput")
    dist = nc.dram_tensor("dist", [SHARD, C], F32, kind="ExternalOutput")

    with tile.TileContext(nc) as tc:
        from contextlib import ExitStack

        with ExitStack() as outer:
            singles = outer.enter_context(tc.tile_pool(name="singles", bufs=1))

            # ---- persistent tiles ----
            node_sb = singles.tile([128, SHARD // 2], mm_dt)    # 16 KiB/part
            cT = singles.tile([128, C], mm_dt)                  # c_hat^T, duplicated at part 64
            ident = singles.tile([128, 128], F32)
            wt_sb = singles.tile([D, D], F32)
            b_bc = singles.tile([128, D], F32)
            if apply_mask:
                mask_sb = singles.tile([128, NTILES], F32)

            nc.sync.dma_start(out=node_sb, in_=node_p[:, :])
            nc.sync.dma_start(out=wt_sb, in_=wt[:, :])
            nc.gpsimd.dma_start(
                out=b_bc, in_=bass.AP(tensor=bvec, offset=0, ap=[[0, 128], [1, D]])
            )
            if apply_mask:
                nc.sync.dma_start(out=mask_sb, in_=maskc[:, :])
            make_identity(nc, ident)

            # ================= centroid prep =================
            with ExitStack() as prep:
                pp = prep.enter_context(tc.tile_pool(name="prep", bufs=1))
                pps = prep.enter_context(
                    tc.tile_pool(name="prep_ps", bufs=2, space="PSUM")
                )

                cw_all = pp.tile([128, 8, D], F32)
                nc.sync.dma_start(
                    out=cw_all, in_=cw[:, :].rearrange("(r p) d -> p r d", p=128)
                )

                sq = pp.tile([128, 8, D - 1], F32)
                nc.vector.tensor_mul(sq, cw_all[:, :, 1:], cw_all[:, :, 1:])
                nrm2 = pp.tile([128, 8], F32)
                nc.vector.tensor_reduce(
                    nrm2, sq, axis=mybir.AxisListType.X, op=ALU.add
                )
                nrm2c = pp.tile([128, 8], F32)
                nc.vector.tensor_scalar_max(nrm2c, nrm2, EPS)
                # n = sqrt(nrm2c) = exp(0.5*ln(nrm2c)); keeps prep on one table set
                lg = pp.tile([128, 8], F32)
                nc.scalar.activation(lg, nrm2c, AF.Ln)
                nvec = pp.tile([128, 8], F32)
                nc.scalar.activation(nvec, lg, AF.Exp, scale=0.5)
                e1 = pp.tile([128, 8], F32)
                nc.scalar.activation(e1, nvec, AF.Exp)
                e2 = pp.tile([128, 8], F32)
                nc.scalar.activation(e2, nvec, AF.Exp, scale=-1.0)
                coshn = pp.tile([128, 8], F32)
                nc.vector.tensor_add(coshn, e1, e2)
                nc.vector.tensor_scalar_mul(coshn, coshn, 0.5)
                rn = pp.tile([128, 8], F32)
                nc.vector.reciprocal(rn, nvec)
                sdiff = pp.tile([128, 8], F32)
                nc.vector.tensor_sub(sdiff, e1, e2)
                fall = pp.tile([128, 8], F32)
                # fall = (0.5 * sdiff) * rn  == sinh(n)/n
                nc.vector.scalar_tensor_tensor(
                    fall, sdiff, 0.5, rn, op0=ALU.mult, op1=ALU.mult
                )

                y_all = pp.tile([128, 8, D], F32)
                t0_in = pp.tile([128, 8], F32)
                for r in range(8):
                    pt = pp.tile([128, D], F32, tag="pt")
                    nc.vector.tensor_copy(pt[:, 0:1], coshn[:, r : r + 1])
                    nc.vector.tensor_scalar_mul(
                        pt[:, 1:], cw_all[:, r, 1:], fall[:, r : r + 1]
                    )
                    ptT_ps = pps.tile([64, 128], F32, tag="ptT_ps")
                    nc.tensor.transpose(ptT_ps, pt, ident)
                    ptT = pp.tile([64, 128], F32, tag="ptT")
                    nc.vector.tensor_copy(ptT, ptT_ps)
                    y_ps = pps.tile([128, D], F32, tag="y_ps")
                    nc.tensor.matmul(y_ps, ptT, wt_sb, start=True, stop=True)
                    nc.vector.tensor_add(y_all[:, r, :], y_ps, b_bc)

                sq2 = pp.tile([128, 8, D - 1], F32)
                nc.vector.tensor_mul(sq2, y_all[:, :, 1:], y_all[:, :, 1:])
                s2 = pp.tile([128, 8], F32)
                nc.vector.tensor_reduce(s2, sq2, axis=mybir.AxisListType.X, op=ALU.add)
                # t0 = sqrt(1 + s2) = exp(0.5*ln(s2 + 1))
                nc.scalar.activation(t0_in, s2, AF.Ln, bias=1.0)
                t0 = pp.tile([128, 8], F32)
                nc.scalar.activation(t0, t0_in, AF.Exp, scale=0.5)

                ch_all = pp.tile([128, 8, D], F32)
                nc.vector.tensor_copy(ch_all[:, :, 0:1], t0)
                nc.vector.tensor_scalar_mul(ch_all[:, :, 1:], y_all[:, :, 1:], -1.0)
                # stage all 8 transposed blocks in one PSUM region, then land
                # cT with a single copy so downstream matmuls wait on one inst
                chT_all = pps.tile([64, 8, 128], F32, tag="chT_all")
                for r in range(8):
                    nc.tensor.transpose(chT_all[:, r, :], ch_all[:, r, :], ident)
                nc.vector.tensor_copy(cT[0:64, :], chT_all)
                # duplicate c_hat^T into partitions 64..127 so matmuls for the
                # second half of the node slab see matching base partitions
                nc.sync.dma_start(out=cT[64:128, :], in_=cT[0:64, :])

            # ================= main loop =================
            # per tile: PE mm -> x (PSUM); DVE: xe = max(x, 1+eps) (clamp +
            # eviction to SBUF); square on GpSimd (mostly) / ACT (some pairs);
            # ACT: s = sqrt(z-1); DVE: t = x + s; ACT: d = ln(t); DMA out.
            # Tiles are processed in PSUM-pairs (2 node tiles = 4 banks) and
            # SBUF-quads (4 node tiles) to amortize per-instruction init.
            with ExitStack() as main:
                xs = main.enter_context(
                    tc.tile_pool(name="x_ps", bufs=2, space="PSUM")
                )
                zs = main.enter_context(tc.tile_pool(name="zs", bufs=2))
                ss = main.enter_context(tc.tile_pool(name="ss", bufs=2))
                ts_pool = main.enter_context(
                    tc.tile_pool(name="ts", bufs=max(2, CHUNK // 4))
                )
                xes = (
                    main.enter_context(tc.tile_pool(name="xes", bufs=3))
                    if clamp
                    else None
                )
                if apply_mask:
                    ds_pool = main.enter_context(tc.tile_pool(name="ds", bufs=2))

                dist_v = dist[:, :].rearrange("(a b p) c -> a p b c", b=4, p=128)

                last_ln = None
                i0 = 0
                while i0 < NTILES:
                    nch = min(CHUNK, NTILES - i0)
                    assert nch % 4 == 0
                    tquads = []
                    first_q = None
                    last_q = None
                    for jp in range(nch // 2):      # jp: pair index in chunk
                        p_glob = (i0 // 2) + jp
                        i_lo = i0 + 2 * jp          # first tile of the pair

                        x_ps = xs.tile([128, 2, C], F32, tag="x")
                        for u in range(2):
                            i = i_lo + u
                            half, col = (
                                (0, i * 128) if i < 32 else (64, (i - 32) * 128)
                            )
                            lhsT = node_sb[half : half + 64, col : col + 128]
                            for bk in range(2):
                                nc.tensor.matmul(
                                    x_ps[:, u, bk * 512 : (bk + 1) * 512],
                                    lhsT,
                                    cT[half : half + 64, bk * 512 : (bk + 1) * 512],
                                    start=True,
                                    stop=True,
                                )

                        if jp % 2 == 0:
                            t_quad = ts_pool.tile([128, 4, C], F32, tag="t")
                            tquads.append((t_quad, i_lo))
                        h2 = (jp % 2) * 2           # quad slot for this pair

                        z_pair = zs.tile([128, 2, C], F32, tag="z")
                        s_pair = ss.tile([128, 2, C], F32, tag="s")

                        xp_flat = x_ps.rearrange("p a c -> p (a c)")
                        if clamp:
                            xe_pair = xes.tile([128, 2, C], F32, tag="xe")
                            xe_flat = xe_pair.rearrange("p a c -> p (a c)")
                            nc.vector.tensor_scalar_max(
                                xe_flat, xp_flat, 1.0 + EPS
                            )
                            xin = xe_flat
                        else:
                            xin = xp_flat
                        zv = z_pair.rearrange("p a c -> p (a c)")
                        sv = s_pair.rearrange("p a c -> p (a c)")
                        tv = t_quad[:, h2 : h2 + 2, :].rearrange("p a c -> p (a c)")

                        qs = nc.scalar.activation(zv, xin, AF.Square)
                        if first_q is None:
                            first_q = qs
                        last_q = nc.scalar.activation(sv, zv, AF.Sqrt, bias=-1.0)
                        nc.vector.tensor_add(tv, xin, sv)

                    if last_ln is not None:
                        # keep ACT in sqrt-phase order after previous ln-phase
                        add_dep_helper(first_q.ins, last_ln.ins, sync=False)

                    for t_quad, i_lo in tquads:
                        quad = i_lo // 4
                        tf = t_quad.rearrange("p a c -> p (a c)")
                        if apply_mask:
                            d4 = ds_pool.tile([128, 4, C], F32, tag="d")
                            li = nc.scalar.activation(
                                d4.rearrange("p a c -> p (a c)"), tf, AF.Ln
                            )
                            for h in range(4):
                                nc.gpsimd.tensor_scalar_mul(
                                    t_quad[:, h, :],
                                    d4[:, h, :],
                                    mask_sb[:, i_lo + h : i_lo + h + 1],
                                )
                        else:
                            # ln in place: t_quad <- ln(t_quad)
                            li = nc.scalar.activation(tf, tf, AF.Ln)
                        add_dep_helper(li.ins, last_q.ins, sync=False)
                        last_ln = li
                        nc.sync.dma_start(out=dist_v[quad], in_=t_quad)

                    i0 += nch

    nc.finalize()
    return nc


def _get_program(apply_mask: bool, clamp: bool) -> bass.Bass:
    key = (apply_mask, clamp, CHUNK, SQ_ACT_EVERY, MM_DTYPE)
    if key not in _PROGRAMS:
        _PROGRAMS[key] = _build(apply_mask, clamp)
    return _PROGRAMS[key]


def _round_f32r(x):
    import ml_dtypes

    hi = x.astype(ml_dtypes.bfloat16).astype(np.float32)
    lo = (x - hi).astype(ml_dtypes.bfloat16).astype(np.float32)
    return (hi + lo).astype(np.float32)


def kernel(node_repr, mask, centroid_weight, W, b):
    global LAST_EXEC_TIME_NS

    node = np.ascontiguousarray(np.asarray(node_repr, dtype=np.float32))
    if MM_DTYPE == "f32r":
        node = _round_f32r(node)
    mask_np = np.ascontiguousarray(np.asarray(mask, dtype=np.float32)).reshape(
        NODE_NUM, 1
    )
    cw_np = np.ascontiguousarray(np.asarray(centroid_weight, dtype=np.float32))
    w_np = np.asarray(W, dtype=np.float32)
    b_np = np.ascontiguousarray(np.asarray(b, dtype=np.float32)).reshape(1, D)
    wt_np = np.ascontiguousarray(w_np.T)

    apply_mask = not bool(np.all(mask_np == 1.0))

    nc = _get_program(apply_mask, False)

    in_maps = []
    for k in range(N_CORES):
        nt = node[k * SHARD : (k + 1) * SHARD, :].T  # [64, 8192]
        node_p = np.ascontiguousarray(
            np.concatenate([nt[:, : SHARD // 2], nt[:, SHARD // 2 :]], axis=0)
        )
        im = {"node_p": node_p, "cw": cw_np, "wt": wt_np, "bvec": b_np}
        if apply_mask:
            im["maskc"] = np.ascontiguousarray(
                mask_np[k * SHARD : (k + 1) * SHARD, 0].reshape(NTILES, 128).T
            )
        in_maps.append(im)

    trace = bool(int(os.environ.get("CD_TRACE", "0")))
    res = run_bass_kernel_spmd(nc, in_maps, list(range(N_CORES)), trace=trace)
    LAST_EXEC_TIME_NS = res.exec_time_ns

    out = np.concatenate([r["dist"] for r in res.results], axis=0)
    return out.astype(np.float32, copy=False)
